# revision 1
# baseline (speedup 1.0000x reference)
"""Trainium2 Bass kernel for the CRW palindrome-walk contrastive loss.

Reference computation (per batch b):
  f = L2-normalize(feats, axis=C)
  A_t = f_t^T f_{t+1}                      [N,N], t = 0..T-2
  R_t = rowsoftmax(A_t / tau)              (right edges)
  L_t = rowsoftmax(A_t^T / tau)            (left edges)
  for i in 1..T-3:
    path_i = R_0 R_1 .. R_i L_i L_{i-1} .. L_0
    loss_i = -mean_n log_softmax(log(path_i + EPS))[n, n]
  loss = mean_i loss_i

Device algorithm (per core, B/8 = 2 batches):
  * Track Q_i = (R_0..R_i)^T and S_i = L_i..L_0.  Both recurrences use
    naturally-stored operands with the PE's lhsT convention:
       Q_i = matmul(lhsT=R_i,    rhs=Q_{i-1})   (= R_i^T @ Q_{i-1})
       S_i = matmul(lhsT=L_i^T,  rhs=S_{i-1})   (= L_i   @ S_{i-1})
  * R_t  = rowsoftmax(E_t)  with E_t = exp(A_t/tau)      [row scale]
  * L_t^T = colsoftmax(E_t)                               [col scale]
    (colsums via ones-matrix matmul, which also broadcasts across partitions)
  * t=0 is symmetric with E'_0 = exp(A_0^T/tau):
       S_0 = L_0 = rowsoftmax(E'_0),  Q_0 = R_0^T = colsoftmax(E'_0)
  * diag(path_i) = colsum_k(Q_i * S_i) -- no transposes anywhere.
  * Rows of path_i sum to exactly 1 (product of stochastic matrices), so
    log_softmax(log(path+EPS)) diag == log(diag + EPS) - log1p(N*EPS);
    the constant is ~1e-17 and is dropped.
  * Each core returns the [1, N] vector of summed log-diagonals over its
    (i, b); the host sums across cores in float64 and scales.
"""

import threading

import numpy as np

import concourse.bass as bass  # noqa: F401  (engine types come via nc)
import concourse.tile as tile
import concourse.mybir as mybir
from concourse import bacc
from concourse.bass_utils import run_bass_kernel_spmd

B, C, T, N = 16, 128, 8, 1024
NCORES = 8
BPC = B // NCORES          # batches per core
TEMP = 0.07
EPS = 1e-20

F32 = mybir.dt.float32
F32R = mybir.dt.float32r
EXP = mybir.ActivationFunctionType.Exp
LN = mybir.ActivationFunctionType.Ln


def _r(ap):
    """View an fp32 AP as float32r for full-rate PE matmuls."""
    return ap.bitcast(F32R)


def build(n=N, t_len=T, bpc=BPC, n_cores=NCORES, passes=1):
    """Build + compile the per-core Bass program.  Parameterized so tests
    can build a shrunken version for CoreSim.  passes>1 repeats the whole
    computation (timing instrumentation only — output is scaled)."""
    NB = n // 128            # partition blocks per matrix dim
    CHN = 512 if n >= 512 else n
    NCH = n // CHN           # 512-wide chunks per matrix dim
    n_steps = t_len - 2      # walk steps i = 1..n_steps

    nc = bacc.Bacc("TRN2", target_bir_lowering=False, debug=False,
                   num_devices=n_cores)
    # Register EPS as a const AP so `activation(..., bias=EPS)` can use it.
    eps_t = nc.alloc_sbuf_tensor("const-eps", [128, 1], F32)
    nc.gpsimd.memset(eps_t.ap(), EPS)
    nc.const_aps.aps[(F32, EPS)] = eps_t.ap()
    nc.all_engine_barrier()
    feats_d = nc.dram_tensor("feats", [bpc, C, t_len, n], F32,
                             kind="ExternalInput")
    out_d = nc.dram_tensor("out", [1, n], F32, kind="ExternalOutput")
    feats_ap = feats_d.ap()
    out_ap = out_d.ap()

    with tile.TileContext(nc) as tc:
        with (
            # SBUF pools (per-partition bytes in comments)
            tc.tile_pool(name="const", bufs=1) as const_pool,            # ones: 512B
            tc.tile_pool(name="slice", bufs=1) as slice_pool,    # raw f: 1x4K
            tc.tile_pool(name="fh", bufs=2) as fh_pool,          # fhat: 2x4K
            tc.tile_pool(name="e", bufs=2 * NB) as e_pool,       # 16x4K
            tc.tile_pool(name="q", bufs=3 * NB) as q_pool,       # 24x2K
            tc.tile_pool(name="s", bufs=3 * NB) as s_pool,       # 24x2K
            tc.tile_pool(name="d", bufs=2) as d_pool,            # 2x2K
            tc.tile_pool(name="cs", bufs=2) as cs_pool,          # nrm 2x2K
            tc.tile_pool(name="stat", bufs=2 * NB) as stat_pool, # [128,1]s
            tc.tile_pool(name="lg", bufs=2) as lg_pool,          # [1,CHN]
            tc.tile_pool(name="acc", bufs=1) as acc_pool,                # [1,n]
            # PSUM pools -- 8 banks total
            tc.tile_pool(name="aps", bufs=2, space="PSUM") as aps,    # 4 banks
            tc.tile_pool(name="qps", bufs=1, space="PSUM") as qps,    # 1
            tc.tile_pool(name="sps", bufs=1, space="PSUM") as sps,    # 1
            tc.tile_pool(name="csps", bufs=2, space="PSUM") as csps,  # 2
        ):
            ones_raw = const_pool.tile([128, 128], F32, tag="ones_raw")
            nc.vector.memset(ones_raw[:], 1.0)
            ones = const_pool.tile([128, 128], F32R, tag="ones")
            nc.scalar.copy(ones[:], ones_raw[:])
            loss_acc = acc_pool.tile([1, n], F32, tag="acc")
            nc.vector.memset(loss_acc[:], 0.0)

            def chs(ch):
                return slice(ch * CHN, (ch + 1) * CHN)

            def load_slice(b, t):
                """DMA feats[b,:,t,:] then L2-normalize columns -> fhat."""
                f = slice_pool.tile([128, n], F32, tag="fraw")
                nc.sync.dma_start(f[:], feats_ap[b, :, t, :])
                fh = fh_pool.tile([128, n], F32R, tag="fh")
                for ch in range(NCH):
                    sq = d_pool.tile([128, CHN], F32R, tag="d")
                    nc.scalar.square(sq[:], f[:, chs(ch)])
                    nps = csps.tile([128, CHN], F32, tag="cps")
                    nc.tensor.matmul(nps[:], _r(ones[:]), _r(sq[:]),
                                     start=True, stop=True)
                    nrm = cs_pool.tile([128, CHN], F32, tag="cs")
                    nc.scalar.sqrt(nrm[:], nps[:])
                    nc.vector.tensor_scalar_add(nrm[:], nrm[:], 1e-12)
                    nc.vector.reciprocal(nrm[:], nrm[:])
                    nc.vector.tensor_mul(fh[:, chs(ch)], f[:, chs(ch)], nrm[:])
                return fh

            def softmax_pair(t, fL, fR, want_cr=True):
                """Affinity + exp for one timestep.

                Returns (e_t[NB] E tiles [128,n],
                         rr[NB]  [128,1] 1/rowsum(E)  (row-softmax scale),
                         cr[NB]  [128,1] 1/colsum(E) per output block
                                 (col-softmax scale, partition-wise)).
                The softmax scales are never applied to E; they are folded
                into the walk (rr: rhs row prescale; cr: S-evac scale).
                """
                e_t, rs_t = [], []
                for nb in range(NB):
                    a_ps = aps.tile([128, n], F32, tag="aps")
                    for ch in range(NCH):
                        nc.tensor.matmul(
                            a_ps[:, chs(ch)],
                            fL[:, nb * 128:(nb + 1) * 128],
                            fR[:, chs(ch)],
                            start=True, stop=True)
                    e = e_pool.tile([128, n], F32R, tag="e")
                    rs = stat_pool.tile([128, 1], F32, tag="rs")
                    nc.scalar.activation(e[:], a_ps[:], EXP,
                                         scale=1.0 / TEMP, accum_out=rs[:])
                    e_t.append(e)
                    rs_t.append(rs)

                rr = []
                for nb in range(NB):
                    r = stat_pool.tile([128, 1], F32, tag="rr")
                    nc.vector.reciprocal(r[:], rs_t[nb][:])
                    rr.append(r)

                cr = []
                if want_cr:
                    # c[m] = sum_k E[k, m], as a per-partition vector per
                    # output block: lhsT = E[:, mb-block], rhs = ones column.
                    # (fp32r matmul requires moving free dim >= 2)
                    for mb in range(NB):
                        mbs = slice(mb * 128, (mb + 1) * 128)
                        c_ps = csps.tile([128, 2], F32, tag="cps")
                        for kb in range(NB):
                            nc.tensor.matmul(c_ps[:], e_t[kb][:, mbs],
                                             ones[:, 0:2],
                                             start=(kb == 0),
                                             stop=(kb == NB - 1))
                        c = stat_pool.tile([128, 1], F32, tag="cr")
                        nc.vector.reciprocal(c[:], c_ps[:, 0:1])
                        cr.append(c)
                return e_t, rr, cr

            def t0_setup(f0, f1):
                """Seed both chains from E' = exp(A_0^T / tau).

                S_0 = rowsoftmax(E') materialized; Q_0 = colsoftmax(E') is
                kept RAW (= E' tiles) with its column scale 1/c'[j] folded
                into the loss: acc[j] -= n_steps * log(c'[j])."""
                e_t, rr, _ = softmax_pair(0, fL=f1, fR=f0, want_cr=False)
                s_cur = [[None] * NCH for _ in range(NB)]
                for nb in range(NB):
                    for ch in range(NCH):
                        s = s_pool.tile([128, CHN], F32R, tag="s")
                        nc.vector.tensor_scalar_mul(
                            s[:], e_t[nb][:, chs(ch)], rr[nb][:])
                        s_cur[nb][ch] = s[:]
                for ch in range(NCH):
                    c_ps = csps.tile([1, CHN], F32, tag="cps")
                    for kb in range(NB):
                        nc.tensor.matmul(c_ps[:], ones[:, 0:1],
                                         e_t[kb][:, chs(ch)],
                                         start=(kb == 0), stop=(kb == NB - 1))
                    lgc = lg_pool.tile([1, CHN], F32, tag="lg")
                    nc.scalar.activation(lgc[:], c_ps[:], LN)
                    nc.vector.tensor_scalar(
                        out=lgc[:], in0=lgc[:], scalar1=-float(n_steps),
                        scalar2=None, op0=mybir.AluOpType.mult)
                    nc.vector.tensor_add(loss_acc[:, chs(ch)],
                                         loss_acc[:, chs(ch)], lgc[:])
                q_cur = [[e_t[nb][:, chs(ch)] for ch in range(NCH)]
                         for nb in range(NB)]
                return q_cur, s_cur

            def walk_step(e_t, rr, cr, q_prev, s_prev):
                """One palindrome step: extend both chains, add diag loss.

                Both chain matmuls take lhsT = E_t directly; the row-softmax
                scale rr is pre-applied to Q_{i-1} rows (contraction side),
                and the col-softmax scale cr is applied on the S evacuation
                (output rows)."""
                for kb in range(NB):
                    for ch in range(NCH):
                        nc.vector.tensor_scalar_mul(
                            q_prev[kb][ch], q_prev[kb][ch], rr[kb][:])
                q_new = [[None] * NCH for _ in range(NB)]
                s_new = [[None] * NCH for _ in range(NB)]
                for ch in range(NCH):
                    for mb in range(NB):
                        mbs = slice(mb * 128, (mb + 1) * 128)
                        qp = qps.tile([128, CHN], F32, tag="qps")
                        for kb in range(NB):
                            nc.tensor.matmul(qp[:], e_t[kb][:, mbs],
                                             q_prev[kb][ch],
                                             start=(kb == 0),
                                             stop=(kb == NB - 1))
                        qn = q_pool.tile([128, CHN], F32R, tag="q")
                        nc.vector.tensor_copy(qn[:], qp[:])
                        q_new[mb][ch] = qn[:]

                        sp = sps.tile([128, CHN], F32, tag="sps")
                        for kb in range(NB):
                            nc.tensor.matmul(sp[:], e_t[kb][:, mbs],
                                             s_prev[kb][ch],
                                             start=(kb == 0),
                                             stop=(kb == NB - 1))
                        sn = s_pool.tile([128, CHN], F32R, tag="s")
                        nc.vector.tensor_scalar_mul(sn[:], sp[:], cr[mb][:])
                        s_new[mb][ch] = sn[:]

                # diag(path) = colsum_k (Q * S); log; accumulate
                for ch in range(NCH):
                    d_ps = csps.tile([1, CHN], F32, tag="cps")
                    for kb in range(NB):
                        d = d_pool.tile([128, CHN], F32R, tag="d")
                        nc.vector.tensor_mul(d[:], q_new[kb][ch],
                                             s_new[kb][ch])
                        nc.tensor.matmul(d_ps[:], ones[:, 0:1], d[:],
                                         start=(kb == 0), stop=(kb == NB - 1))
                    lg = lg_pool.tile([1, CHN], F32, tag="lg")
                    nc.scalar.activation(lg[:], d_ps[:], LN, bias=EPS)
                    nc.vector.tensor_add(loss_acc[:, chs(ch)],
                                         loss_acc[:, chs(ch)], lg[:])
                return q_new, s_new

            for b in [bb for _ in range(passes) for bb in range(bpc)]:
                f0 = load_slice(b, 0)
                f1 = load_slice(b, 1)
                q_cur, s_cur = t0_setup(f0, f1)
                f_prev = f1
                for t in range(1, n_steps + 1):
                    f_next = load_slice(b, t + 1)
                    e_t, rr, cr = softmax_pair(t, fL=f_prev, fR=f_next)
                    q_cur, s_cur = walk_step(e_t, rr, cr, q_cur, s_cur)
                    f_prev = f_next

            nc.sync.dma_start(out_ap[:, :], loss_acc[:])

    nc.compile()
    return nc


_build_lock = threading.Lock()
_built_nc = None


def _get_nc():
    global _built_nc
    with _build_lock:
        if _built_nc is None:
            _built_nc = build()
    return _built_nc


LAST_RESULT = None  # BassKernelResults of the most recent run (for profiling)


def kernel(feats: np.ndarray) -> np.ndarray:
    global LAST_RESULT
    feats = np.ascontiguousarray(np.asarray(feats), dtype=np.float32)
    assert feats.shape == (B, C, T, N), feats.shape
    nc = _get_nc()
    in_maps = [
        {"feats": np.ascontiguousarray(feats[c * BPC:(c + 1) * BPC])}
        for c in range(NCORES)
    ]
    res = run_bass_kernel_spmd(nc, in_maps, core_ids=list(range(NCORES)))
    LAST_RESULT = res
    total = 0.0
    for r in res.results:
        total += r["out"].astype(np.float64).sum()
    n_walks = T - 2  # i = 1..T-2 inclusive
    loss = -total / (n_walks * B * N)
    return np.float32(loss)



# revision 9
# speedup vs baseline: 1.3489x; 1.3489x over previous
"""Trainium2 Bass kernel for the CRW palindrome-walk contrastive loss.

Reference computation (per batch b):
  f = L2-normalize(feats, axis=C)
  A_t = f_t^T f_{t+1}                      [N,N], t = 0..T-2
  R_t = rowsoftmax(A_t / tau)              (right edges)
  L_t = rowsoftmax(A_t^T / tau)            (left edges)
  for i in 1..T-2:
    path_i = R_0 R_1 .. R_i L_i L_{i-1} .. L_0
    loss_i = -mean_n log_softmax(log(path_i + EPS))[n, n]
  loss = mean_i loss_i

Device algorithm (per core, B/8 = 2 batches), v2 restructured for engine
overlap (the fp32r baseline measured 1.32 ms with VectorE 52% busy and
the PE HAM-throttled 35% of the time):

  * All chain operands are bf16 (PE streams bf16 at the same 1 col/cycle
    as fp32r, FWL halves weight-load time, DVE ops on bf16 SBUF tiles run
    in perf mode, SBUF footprint halves).
  * Track Q_i = (R_0..R_i)^T and S_i = L_i..L_0 as in the baseline:
       Q_i = matmul(lhsT=E_i, rhs=rr_i*Q_{i-1})   (rr prescale pass)
       S_i = matmul(lhsT=E_i, rhs=S_{i-1})        (cr scale at evacuation)
    with E_t = exp(A_t/tau) kept UNnormalized; the row-softmax scale
    rr_t = 1/rowsum and col-softmax scale cr_t = 1/colsum are folded in.
  * cr_t in partition-block form comes from a SECOND affinity pass
    A_t^T = f_{t+1}^T f_t whose exp rowsums are E_t's colsums (replaces
    the baseline's 64 tiny [128,2] colsum matmuls per t).
  * Per walk step the engines split:  PE: chain + affinity + diag
    matmuls; ACT: exp(+rowsum accum) / sqrt; DVE: PSUM evacuations,
    prescales, stats; GpSimd: the diag elementwise products Q*S.
    Emission interleaves softmax(t+1) and diag(t-1) chunks into the
    chain-matmul stream so every engine has continuous work and the PE
    HAM stays at full clock.
  * diag(path_i) = colsum_k(Q_i * S_i); rows of path_i sum to 1 so
    log_softmax reduces to log(diag + EPS).
  * Each core returns the [1, N] vector of summed log-diagonals over
    (i, b); the host sums across cores in float64 and scales.
"""

import threading

import numpy as np

import concourse.bass as bass  # noqa: F401
import concourse.tile as tile
import concourse.mybir as mybir
from concourse import bacc
from concourse.bass_utils import run_bass_kernel_spmd

B, C, T, N = 16, 128, 8, 1024
NCORES = 8
BPC = B // NCORES          # batches per core
TEMP = 0.07
EPS = 1e-20
NB = N // 128              # partition blocks per matrix dim (8)
H = 512                    # PSUM half width (one bank of fp32)
NH = N // H                # halves (2)
NSTEP = T - 2              # walk steps i = 1..6

F32 = mybir.dt.float32
BF16 = mybir.dt.bfloat16
EXP = mybir.ActivationFunctionType.Exp
LN = mybir.ActivationFunctionType.Ln


def build(n_cores=NCORES, bpc=BPC):
    nc = bacc.Bacc("TRN2", target_bir_lowering=False, debug=False,
                   num_devices=n_cores)
    # Register EPS as a const AP so `activation(..., bias=EPS)` can use it.
    eps_t = nc.alloc_sbuf_tensor("const-eps", [128, 1], F32)
    nc.gpsimd.memset(eps_t.ap(), EPS)
    nc.const_aps.aps[(F32, EPS)] = eps_t.ap()
    nc.all_engine_barrier()
    feats_d = nc.dram_tensor("feats", [bpc, C, T, N], F32,
                             kind="ExternalInput")
    out_d = nc.dram_tensor("out", [1, N], F32, kind="ExternalOutput")
    feats_ap = feats_d.ap()
    out_ap = out_d.ap()

    with tile.TileContext(nc) as tc:
        with (
            # SBUF pools (bufs = ring slots per tag; [128,1024] bf16 = 2KB/p)
            tc.tile_pool(name="const", bufs=1) as const_pool,
            tc.tile_pool(name="f", bufs=2) as f_pool,        # raw fp32 slice
            tc.tile_pool(name="sq", bufs=2) as sq_pool,
            tc.tile_pool(name="nrm", bufs=4) as nrm_pool,
            tc.tile_pool(name="fh", bufs=4) as fh_pool,
            tc.tile_pool(name="e", bufs=24) as e_pool,       # E_t bf16
            tc.tile_pool(name="q", bufs=12) as q_pool,       # Q_i bf16
            tc.tile_pool(name="qh", bufs=16) as qh_pool,     # rr-prescaled Q
            tc.tile_pool(name="s", bufs=16) as s_pool,       # S_i bf16
            tc.tile_pool(name="d", bufs=8) as d_pool,        # Q*S bf16
            tc.tile_pool(name="esc", bufs=2) as esc_pool,    # A^T exp scratch
            tc.tile_pool(name="st", bufs=4) as st_pool,      # [128,NB] stats
            tc.tile_pool(name="lg", bufs=2) as lg_pool,
            tc.tile_pool(name="acc", bufs=1) as acc_pool,
            # PSUM: ps ring 4x[128,512] (affinity/norm/diag) + chain 2+2
            tc.tile_pool(name="ps", bufs=4, space="PSUM") as ps_pool,
            tc.tile_pool(name="qps", bufs=1, space="PSUM") as qps_pool,
            tc.tile_pool(name="sps", bufs=1, space="PSUM") as sps_pool,
        ):
            ones_raw = const_pool.tile([128, 128], F32, tag="ones_raw")
            nc.vector.memset(ones_raw[:], 1.0)
            ones = const_pool.tile([128, 128], BF16, tag="ones")
            nc.vector.tensor_copy(ones[:], ones_raw[:])
            acc = acc_pool.tile([1, N], F32, tag="acc")
            nc.vector.memset(acc[:], 0.0)

            def hsl(h):
                return slice(h * H, (h + 1) * H)

            def bsl(nb):
                return slice(nb * 128, (nb + 1) * 128)

            def load_fhat(b, t):
                """DMA feats[b,:,t,:]; L2-normalize columns -> bf16 fhat."""
                f = f_pool.tile([128, N], F32, tag="f")
                nc.sync.dma_start(f[:], feats_ap[b, :, t, :])
                fh = fh_pool.tile([128, N], BF16, tag="fh")
                for h in range(NH):
                    sq = sq_pool.tile([128, H], BF16, tag="sq")
                    nc.scalar.square(sq[:], f[:, hsl(h)])
                    nps = ps_pool.tile([128, H], F32, tag="ps")
                    nc.tensor.matmul(nps[:], ones[:], sq[:],
                                     start=True, stop=True)
                    nrm = nrm_pool.tile([128, H], F32, tag="nrm")
                    nc.scalar.sqrt(nrm[:], nps[:])
                    nc.vector.reciprocal(nrm[:], nrm[:])
                    nc.vector.tensor_mul(fh[:, hsl(h)], f[:, hsl(h)], nrm[:])
                return fh

            def affinity_chunk(fhL, fhR, nb, e_dst, acc_dst):
                """One nb block of E = exp((fhL^T fhR)/tau) [128, N] bf16,
                as two 512-halves through 1-bank PSUM tiles.  The rowsum
                half-accumulators land in acc_dst[h][:, nb]."""
                for h in range(NH):
                    a_ps = ps_pool.tile([128, H], F32, tag="ps")
                    nc.tensor.matmul(a_ps[:], fhL[:, bsl(nb)],
                                     fhR[:, hsl(h)], start=True, stop=True)
                    nc.scalar.activation(
                        e_dst[:, hsl(h)], a_ps[:], EXP,
                        scale=1.0 / TEMP,
                        accum_out=acc_dst[h][:, nb:nb + 1])

            def softmax_emit(fhL, fhR, nb_pair, e_tiles, rsh, csh):
                """Emit affinity+exp for blocks nb_pair of both A (-> E
                tiles, rowsums rsh) and A^T (-> scratch, rowsums csh =
                colsums of E)."""
                for nb in nb_pair:
                    affinity_chunk(fhL, fhR, nb, e_tiles[nb], rsh)
                for nb in nb_pair:
                    esc = esc_pool.tile([128, N], BF16, tag="esc")
                    affinity_chunk(fhR, fhL, nb, esc, csh)

            def stats_finish(rsh, csh):
                """rr = 1/(rsh0+rsh1), cr = 1/(csh0+csh1), both [128, NB]."""
                rr = st_pool.tile([128, NB], F32, tag="rr")
                nc.vector.tensor_add(rr[:], rsh[0][:], rsh[1][:])
                nc.vector.reciprocal(rr[:], rr[:])
                cr = st_pool.tile([128, NB], F32, tag="cr")
                nc.vector.tensor_add(cr[:], csh[0][:], csh[1][:])
                nc.vector.reciprocal(cr[:], cr[:])
                return rr, cr

            def new_stat_halves(tag):
                return [st_pool.tile([128, NB], F32, tag=f"{tag}{h}",
                                      name=f"st_{tag}{h}")
                        for h in range(NH)]

            def chain_mb(e_tiles, qh_prev, s_prev, mb):
                """Q[mb] then S[mb]: each 16 accumulating matmuls into a
                2-bank PSUM tile, evacuated by DVE to bf16 SBUF."""
                qp = qps_pool.tile([128, N], F32, tag="qps")
                for kb in range(NB):
                    for h in range(NH):
                        nc.tensor.matmul(qp[:, hsl(h)],
                                         e_tiles[kb][:, bsl(mb)],
                                         qh_prev[kb][:, hsl(h)],
                                         start=(kb == 0), stop=(kb == NB - 1))
                qn = q_pool.tile([128, N], BF16, tag="q")
                nc.vector.tensor_copy(qn[:], qp[:])

                sp = sps_pool.tile([128, N], F32, tag="sps")
                for kb in range(NB):
                    for h in range(NH):
                        nc.tensor.matmul(sp[:, hsl(h)],
                                         e_tiles[kb][:, bsl(mb)],
                                         s_prev[kb][:, hsl(h)],
                                         start=(kb == 0), stop=(kb == NB - 1))
                return qn, sp

            def diag_half(d_tiles, h):
                """colsum_k(Q*S) for one 512-half; LN(+EPS) into acc."""
                d_ps = ps_pool.tile([1, H], F32, tag="ps")
                for kb in range(NB):
                    nc.tensor.matmul(d_ps[:], ones[:, 0:1],
                                     d_tiles[kb][:, hsl(h)],
                                     start=(kb == 0), stop=(kb == NB - 1))
                lg = lg_pool.tile([1, H], F32, tag="lg")
                nc.scalar.activation(lg[:], d_ps[:], LN, bias=EPS)
                nc.vector.tensor_add(acc[:, hsl(h)], acc[:, hsl(h)], lg[:])

            for b in range(bpc):
                # ---------------- prelude: t0 + softmax(1) ----------------
                fh = {0: load_fhat(b, 0), 1: load_fhat(b, 1)}

                # E'_0 = exp(A_0^T/tau): seeds BOTH chains.
                e0 = [e_pool.tile([128, N], BF16, tag="e", name=f"e0_{kb}")
                      for kb in range(NB)]
                rs0h = new_stat_halves("r0h")
                for nb in range(NB):
                    affinity_chunk(fh[1], fh[0], nb, e0[nb], rs0h)

                # Q_0 raw = E'_0; its missing column scale 1/colsum(E'_0)
                # is folded into the loss: acc[j] -= NSTEP*log(colsum[j]).
                for h in range(NH):
                    c_ps = ps_pool.tile([1, H], F32, tag="ps")
                    for kb in range(NB):
                        nc.tensor.matmul(c_ps[:], ones[:, 0:1],
                                         e0[kb][:, hsl(h)],
                                         start=(kb == 0), stop=(kb == NB - 1))
                    lgc = lg_pool.tile([1, H], F32, tag="lg")
                    nc.scalar.activation(lgc[:], c_ps[:], LN)
                    nc.vector.tensor_scalar(
                        out=lgc[:], in0=lgc[:], scalar1=-float(NSTEP),
                        scalar2=None, op0=mybir.AluOpType.mult)
                    nc.vector.tensor_add(acc[:, hsl(h)], acc[:, hsl(h)],
                                         lgc[:])

                # softmax(1): E_1, rr_1, cr_1
                fh[2] = load_fhat(b, 2)
                e_cur = [e_pool.tile([128, N], BF16, tag="e", name=f"e1_{kb}")
                         for kb in range(NB)]
                rsh = new_stat_halves("rsh")
                csh = new_stat_halves("csh")
                softmax_emit(fh[1], fh[2], range(NB), e_cur, rsh, csh)
                rr_cur, cr_cur = stats_finish(rsh, csh)

                # rr'_0 = 1/rowsum(E'_0): S_0 = rowsoftmax(E'_0) = L_0.
                rr0 = st_pool.tile([128, NB], F32, tag="rr0")
                nc.vector.tensor_add(rr0[:], rs0h[0][:], rs0h[1][:])
                nc.vector.reciprocal(rr0[:], rr0[:])
                s_prev = []
                qh_prev = []
                for kb in range(NB):
                    s0 = s_pool.tile([128, N], BF16, tag="s")
                    nc.vector.tensor_scalar_mul(s0[:], e0[kb][:],
                                                rr0[:, kb:kb + 1])
                    s_prev.append(s0)
                    # Q'_0 = rr_1-prescaled raw Q_0
                    q0 = qh_pool.tile([128, N], BF16, tag="qh")
                    nc.vector.tensor_scalar_mul(q0[:], e0[kb][:],
                                                rr_cur[:, kb:kb + 1])
                    qh_prev.append(q0)

                d_pending = None     # d tiles of step i-1 awaiting diag
                # ---------------- walk steps i = 1..6 ----------------
                for i in range(1, NSTEP + 1):
                    last = (i == NSTEP)
                    e_nxt = None
                    rsh_n = csh_n = None
                    if not last:
                        fh[i + 2] = load_fhat(b, i + 2)
                        e_nxt = [e_pool.tile([128, N], BF16, tag="e",
                                            name=f"e_{i}_{kb}")
                                 for kb in range(NB)]
                        rsh_n = new_stat_halves("rsh")
                        csh_n = new_stat_halves("csh")

                    q_new, s_new = [], []
                    for mb in range(NB):
                        qn, sp = chain_mb(e_cur, qh_prev, s_prev, mb)
                        q_new.append(qn)
                        # S evac with cr_i scale
                        sn = s_pool.tile([128, N], BF16, tag="s")
                        nc.vector.tensor_scalar_mul(sn[:], sp[:],
                                                    cr_cur[:, mb:mb + 1])
                        s_new.append(sn)
                        # interleave lookahead softmax + trailing diag into
                        # the PE stream between chain blocks
                        if not last and mb in (0, 2, 4, 6):
                            softmax_emit(fh[i + 1], fh[i + 2],
                                         (mb, mb + 1), e_nxt, rsh_n, csh_n)
                        if d_pending is not None and mb in (2, 5):
                            diag_half(d_pending, 0 if mb == 2 else 1)
                            if mb == 5:
                                d_pending = None

                    # d_i = Q_i * S_i on GpSimd (bf16, SBUF only)
                    d_tiles = []
                    for kb in range(NB):
                        d = d_pool.tile([128, N], BF16, tag="d")
                        nc.gpsimd.tensor_mul(d[:], q_new[kb][:], s_new[kb][:])
                        d_tiles.append(d)

                    if not last:
                        rr_nxt, cr_nxt = stats_finish(rsh_n, csh_n)
                        # prescale Q_i by rr_{i+1} for the next step's rhs
                        qh_new = []
                        for kb in range(NB):
                            qh = qh_pool.tile([128, N], BF16, tag="qh")
                            nc.vector.tensor_scalar_mul(
                                qh[:], q_new[kb][:], rr_nxt[:, kb:kb + 1])
                            qh_new.append(qh)
                        qh_prev = qh_new
                        e_cur = e_nxt
                        rr_cur, cr_cur = rr_nxt, cr_nxt
                    s_prev = s_new
                    d_pending = d_tiles

                # trailing diag for i = 6
                diag_half(d_pending, 0)
                diag_half(d_pending, 1)
                d_pending = None

            nc.sync.dma_start(out_ap[:, :], acc[:])

    nc.compile()
    return nc


_build_lock = threading.Lock()
_built_nc = None


def _get_nc():
    global _built_nc
    with _build_lock:
        if _built_nc is None:
            _built_nc = build()
    return _built_nc


LAST_RESULT = None  # BassKernelResults of the most recent run (for profiling)


def kernel(feats: np.ndarray) -> np.ndarray:
    global LAST_RESULT
    feats = np.ascontiguousarray(np.asarray(feats), dtype=np.float32)
    assert feats.shape == (B, C, T, N), feats.shape
    nc = _get_nc()
    in_maps = [
        {"feats": np.ascontiguousarray(feats[c * BPC:(c + 1) * BPC])}
        for c in range(NCORES)
    ]
    res = run_bass_kernel_spmd(nc, in_maps, core_ids=list(range(NCORES)))
    LAST_RESULT = res
    total = 0.0
    for r in res.results:
        total += r["out"].astype(np.float64).sum()
    loss = -total / (NSTEP * B * N)
    return np.float32(loss)


# revision 21
# speedup vs baseline: 1.3490x; 1.0001x over previous
"""Trainium2 Bass kernel for the CRW palindrome-walk contrastive loss.

Reference computation (per batch b):
  f = L2-normalize(feats, axis=C)
  A_t = f_t^T f_{t+1}                      [N,N], t = 0..T-2
  R_t = rowsoftmax(A_t / tau)              (right edges)
  L_t = rowsoftmax(A_t^T / tau)            (left edges)
  for i in 1..T-2:
    path_i = R_0 R_1 .. R_i L_i L_{i-1} .. L_0
    loss_i = -mean_n log_softmax(log(path_i + EPS))[n, n]
  loss = mean_i loss_i

Device algorithm (per core, B/8 = 2 batches), v2 restructured for engine
overlap (the fp32r baseline measured 1.32 ms with VectorE 52% busy and
the PE HAM-throttled 35% of the time):

  * All chain operands are bf16 (PE streams bf16 at the same 1 col/cycle
    as fp32r, FWL halves weight-load time, DVE ops on bf16 SBUF tiles run
    in perf mode, SBUF footprint halves).
  * Track Q_i = (R_0..R_i)^T and S_i = L_i..L_0 as in the baseline:
       Q_i = matmul(lhsT=E_i, rhs=rr_i*Q_{i-1})   (rr prescale pass)
       S_i = matmul(lhsT=E_i, rhs=S_{i-1})        (cr scale at evacuation)
    with E_t = exp(A_t/tau) kept UNnormalized; the row-softmax scale
    rr_t = 1/rowsum and col-softmax scale cr_t = 1/colsum are folded in.
  * cr_t in partition-block form comes from a SECOND affinity pass
    A_t^T = f_{t+1}^T f_t whose exp rowsums are E_t's colsums (replaces
    the baseline's 64 tiny [128,2] colsum matmuls per t).
  * Per walk step the engines split:  PE: chain + affinity + diag
    matmuls; ACT: exp(+rowsum accum) / sqrt; DVE: PSUM evacuations,
    prescales, stats; GpSimd: the diag elementwise products Q*S.
    Emission interleaves softmax(t+1) and diag(t-1) chunks into the
    chain-matmul stream so every engine has continuous work and the PE
    HAM stays at full clock.
  * diag(path_i) = colsum_k(Q_i * S_i); rows of path_i sum to 1 so
    log_softmax reduces to log(diag + EPS).
  * Each core returns the [1, N] vector of summed log-diagonals over
    (i, b); the host sums across cores in float64 and scales.
"""

import threading

import numpy as np

import concourse.bass as bass  # noqa: F401
import concourse.tile as tile
import concourse.mybir as mybir
from concourse import bacc
from concourse.bass_utils import run_bass_kernel_spmd

B, C, T, N = 16, 128, 8, 1024
NCORES = 8
BPC = B // NCORES          # batches per core
TEMP = 0.07
EPS = 1e-20
NB = N // 128              # partition blocks per matrix dim (8)
H = 512                    # PSUM half width (one bank of fp32)
NH = N // H                # halves (2)
NSTEP = T - 2              # walk steps i = 1..6

F32 = mybir.dt.float32
BF16 = mybir.dt.bfloat16
EXP = mybir.ActivationFunctionType.Exp
LN = mybir.ActivationFunctionType.Ln


def build(n_cores=NCORES, bpc=BPC):
    nc = bacc.Bacc("TRN2", target_bir_lowering=False, debug=False,
                   num_devices=n_cores)
    # Register EPS as a const AP so `activation(..., bias=EPS)` can use it.
    eps_t = nc.alloc_sbuf_tensor("const-eps", [128, 1], F32)
    nc.gpsimd.memset(eps_t.ap(), EPS)
    nc.const_aps.aps[(F32, EPS)] = eps_t.ap()
    nc.all_engine_barrier()
    feats_d = nc.dram_tensor("feats", [bpc, C, T, N], F32,
                             kind="ExternalInput")
    out_d = nc.dram_tensor("out", [1, N], F32, kind="ExternalOutput")
    feats_ap = feats_d.ap()
    out_ap = out_d.ap()

    with tile.TileContext(nc) as tc:
        with (
            # SBUF pools (bufs = ring slots per tag; [128,1024] bf16 = 2KB/p)
            tc.tile_pool(name="const", bufs=1) as const_pool,
            tc.tile_pool(name="f", bufs=2) as f_pool,        # raw fp32 slice
            tc.tile_pool(name="sq", bufs=2) as sq_pool,
            tc.tile_pool(name="nrm", bufs=4) as nrm_pool,
            tc.tile_pool(name="fh", bufs=4) as fh_pool,
            tc.tile_pool(name="e", bufs=24) as e_pool,       # E_t bf16
            tc.tile_pool(name="q", bufs=12) as q_pool,       # Q_i bf16
            tc.tile_pool(name="qh", bufs=16) as qh_pool,     # rr-prescaled Q
            tc.tile_pool(name="s", bufs=16) as s_pool,       # S_i bf16
            tc.tile_pool(name="d", bufs=8) as d_pool,        # Q*S bf16
            tc.tile_pool(name="esc", bufs=2) as esc_pool,    # A^T exp scratch
            tc.tile_pool(name="st", bufs=4) as st_pool,      # [128,NB] stats
            tc.tile_pool(name="lg", bufs=2) as lg_pool,
            tc.tile_pool(name="acc", bufs=1) as acc_pool,
            # PSUM: ps ring 4x[128,512] (affinity/norm/diag) + chain 2+2
            tc.tile_pool(name="ps", bufs=4, space="PSUM") as ps_pool,
            tc.tile_pool(name="qps", bufs=1, space="PSUM") as qps_pool,
            tc.tile_pool(name="sps", bufs=1, space="PSUM") as sps_pool,
        ):
            ones_raw = const_pool.tile([128, 128], F32, tag="ones_raw")
            nc.vector.memset(ones_raw[:], 1.0)
            ones = const_pool.tile([128, 128], BF16, tag="ones")
            nc.vector.tensor_copy(ones[:], ones_raw[:])
            acc = acc_pool.tile([1, N], F32, tag="acc")
            nc.vector.memset(acc[:], 0.0)

            def hsl(h):
                return slice(h * H, (h + 1) * H)

            def bsl(nb):
                return slice(nb * 128, (nb + 1) * 128)

            def load_fhat(b, t):
                """DMA feats[b,:,t,:]; L2-normalize columns -> bf16 fhat."""
                f = f_pool.tile([128, N], F32, tag="f")
                nc.sync.dma_start(f[:], feats_ap[b, :, t, :])
                fh = fh_pool.tile([128, N], BF16, tag="fh")
                for h in range(NH):
                    sq = sq_pool.tile([128, H], BF16, tag="sq")
                    nc.scalar.square(sq[:], f[:, hsl(h)])
                    nps = ps_pool.tile([128, H], F32, tag="ps")
                    nc.tensor.matmul(nps[:], ones[:], sq[:],
                                     start=True, stop=True)
                    nrm = nrm_pool.tile([128, H], F32, tag="nrm")
                    nc.scalar.sqrt(nrm[:], nps[:])
                    nc.vector.reciprocal(nrm[:], nrm[:])
                    nc.vector.tensor_mul(fh[:, hsl(h)], f[:, hsl(h)], nrm[:])
                return fh

            def affinity_chunk(fhL, fhR, nb, e_dst, acc_dst):
                """One nb block of E = exp((fhL^T fhR)/tau) [128, N] bf16,
                as two 512-halves through 1-bank PSUM tiles.  The rowsum
                half-accumulators land in acc_dst[h][:, nb]."""
                for h in range(NH):
                    a_ps = ps_pool.tile([128, H], F32, tag="ps")
                    nc.tensor.matmul(a_ps[:], fhL[:, bsl(nb)],
                                     fhR[:, hsl(h)], start=True, stop=True)
                    nc.scalar.activation(
                        e_dst[:, hsl(h)], a_ps[:], EXP,
                        scale=1.0 / TEMP,
                        accum_out=acc_dst[h][:, nb:nb + 1])

            def softmax_emit(fhL, fhR, nb_pair, e_tiles, rsh, csh):
                """Emit affinity+exp for blocks nb_pair of both A (-> E
                tiles, rowsums rsh) and A^T (-> scratch, rowsums csh =
                colsums of E)."""
                for nb in nb_pair:
                    affinity_chunk(fhL, fhR, nb, e_tiles[nb], rsh)
                for nb in nb_pair:
                    esc = esc_pool.tile([128, N], BF16, tag="esc")
                    affinity_chunk(fhR, fhL, nb, esc, csh)

            def stats_finish(rsh, csh):
                """rr = 1/(rsh0+rsh1), cr = 1/(csh0+csh1), both [128, NB]."""
                rr = st_pool.tile([128, NB], F32, tag="rr")
                nc.vector.tensor_add(rr[:], rsh[0][:], rsh[1][:])
                nc.vector.reciprocal(rr[:], rr[:])
                cr = st_pool.tile([128, NB], F32, tag="cr")
                nc.vector.tensor_add(cr[:], csh[0][:], csh[1][:])
                nc.vector.reciprocal(cr[:], cr[:])
                return rr, cr

            def new_stat_halves(tag):
                return [st_pool.tile([128, NB], F32, tag=f"{tag}{h}",
                                      name=f"st_{tag}{h}")
                        for h in range(NH)]

            def chain_mb(e_tiles, qh_prev, s_prev, mb):
                """Q[mb] then S[mb]: each 16 accumulating matmuls into a
                2-bank PSUM tile, evacuated by DVE to bf16 SBUF."""
                qp = qps_pool.tile([128, N], F32, tag="qps")
                for kb in range(NB):
                    for h in range(NH):
                        nc.tensor.matmul(qp[:, hsl(h)],
                                         e_tiles[kb][:, bsl(mb)],
                                         qh_prev[kb][:, hsl(h)],
                                         start=(kb == 0), stop=(kb == NB - 1))
                qn = q_pool.tile([128, N], BF16, tag="q")
                nc.vector.tensor_copy(qn[:], qp[:])

                sp = sps_pool.tile([128, N], F32, tag="sps")
                for kb in range(NB):
                    for h in range(NH):
                        nc.tensor.matmul(sp[:, hsl(h)],
                                         e_tiles[kb][:, bsl(mb)],
                                         s_prev[kb][:, hsl(h)],
                                         start=(kb == 0), stop=(kb == NB - 1))
                return qn, sp

            def diag_half(d_tiles, h):
                """colsum_k(Q*S) for one 512-half; LN(+EPS) into acc."""
                d_ps = ps_pool.tile([1, H], F32, tag="ps")
                for kb in range(NB):
                    nc.tensor.matmul(d_ps[:], ones[:, 0:1],
                                     d_tiles[kb][:, hsl(h)],
                                     start=(kb == 0), stop=(kb == NB - 1))
                lg = lg_pool.tile([1, H], F32, tag="lg")
                nc.scalar.activation(lg[:], d_ps[:], LN, bias=EPS)
                nc.vector.tensor_add(acc[:, hsl(h)], acc[:, hsl(h)], lg[:])

            for b in range(bpc):
                # ---------------- prelude: t0 + softmax(1) ----------------
                fh = {0: load_fhat(b, 0), 1: load_fhat(b, 1)}

                # E'_0 = exp(A_0^T/tau): seeds BOTH chains.
                e0 = [e_pool.tile([128, N], BF16, tag="e", name=f"e0_{kb}")
                      for kb in range(NB)]
                rs0h = new_stat_halves("r0h")
                for nb in range(NB):
                    affinity_chunk(fh[1], fh[0], nb, e0[nb], rs0h)

                # Q_0 raw = E'_0; its missing column scale 1/colsum(E'_0)
                # is folded into the loss: acc[j] -= NSTEP*log(colsum[j]).
                for h in range(NH):
                    c_ps = ps_pool.tile([1, H], F32, tag="ps")
                    for kb in range(NB):
                        nc.tensor.matmul(c_ps[:], ones[:, 0:1],
                                         e0[kb][:, hsl(h)],
                                         start=(kb == 0), stop=(kb == NB - 1))
                    lgc = lg_pool.tile([1, H], F32, tag="lg")
                    nc.scalar.activation(lgc[:], c_ps[:], LN)
                    nc.vector.tensor_scalar(
                        out=lgc[:], in0=lgc[:], scalar1=-float(NSTEP),
                        scalar2=None, op0=mybir.AluOpType.mult)
                    nc.vector.tensor_add(acc[:, hsl(h)], acc[:, hsl(h)],
                                         lgc[:])

                # softmax(1): E_1, rr_1, cr_1
                fh[2] = load_fhat(b, 2)
                e_cur = [e_pool.tile([128, N], BF16, tag="e", name=f"e1_{kb}")
                         for kb in range(NB)]
                rsh = new_stat_halves("rsh")
                csh = new_stat_halves("csh")
                softmax_emit(fh[1], fh[2], range(NB), e_cur, rsh, csh)
                rr_cur, cr_cur = stats_finish(rsh, csh)

                # rr'_0 = 1/rowsum(E'_0): S_0 = rowsoftmax(E'_0) = L_0.
                rr0 = st_pool.tile([128, NB], F32, tag="rr0")
                nc.vector.tensor_add(rr0[:], rs0h[0][:], rs0h[1][:])
                nc.vector.reciprocal(rr0[:], rr0[:])
                s_prev = []
                qh_prev = []
                for kb in range(NB):
                    s0 = s_pool.tile([128, N], BF16, tag="s")
                    nc.vector.tensor_scalar_mul(s0[:], e0[kb][:],
                                                rr0[:, kb:kb + 1])
                    s_prev.append(s0)
                    # Q'_0 = rr_1-prescaled raw Q_0
                    q0 = qh_pool.tile([128, N], BF16, tag="qh")
                    nc.vector.tensor_scalar_mul(q0[:], e0[kb][:],
                                                rr_cur[:, kb:kb + 1])
                    qh_prev.append(q0)

                d_pending = None     # d tiles of step i-1 awaiting diag
                # ---------------- walk steps i = 1..6 ----------------
                for i in range(1, NSTEP + 1):
                    last = (i == NSTEP)
                    e_nxt = None
                    rsh_n = csh_n = None
                    if not last:
                        fh[i + 2] = load_fhat(b, i + 2)
                        e_nxt = [e_pool.tile([128, N], BF16, tag="e",
                                            name=f"e_{i}_{kb}")
                                 for kb in range(NB)]
                        rsh_n = new_stat_halves("rsh")
                        csh_n = new_stat_halves("csh")

                    q_new, s_new = [], []
                    for mb in range(NB):
                        qn, sp = chain_mb(e_cur, qh_prev, s_prev, mb)
                        q_new.append(qn)
                        # S evac with cr_i scale
                        sn = s_pool.tile([128, N], BF16, tag="s")
                        nc.vector.tensor_scalar_mul(sn[:], sp[:],
                                                    cr_cur[:, mb:mb + 1])
                        s_new.append(sn)
                        # interleave lookahead softmax + trailing diag into
                        # the PE stream between chain blocks
                        if not last and mb in (0, 2, 4, 6):
                            softmax_emit(fh[i + 1], fh[i + 2],
                                         (mb, mb + 1), e_nxt, rsh_n, csh_n)
                        if d_pending is not None and mb in (2, 5):
                            diag_half(d_pending, 0 if mb == 2 else 1)
                            if mb == 5:
                                d_pending = None

                    # d_i = Q_i * S_i on GpSimd (bf16, SBUF only)
                    d_tiles = []
                    for kb in range(NB):
                        d = d_pool.tile([128, N], BF16, tag="d")
                        nc.gpsimd.tensor_mul(d[:], q_new[kb][:], s_new[kb][:])
                        d_tiles.append(d)

                    if not last:
                        rr_nxt, cr_nxt = stats_finish(rsh_n, csh_n)
                        # prescale Q_i by rr_{i+1} for the next step's rhs
                        qh_new = []
                        for kb in range(NB):
                            qh = qh_pool.tile([128, N], BF16, tag="qh")
                            nc.vector.tensor_scalar_mul(
                                qh[:], q_new[kb][:], rr_nxt[:, kb:kb + 1])
                            qh_new.append(qh)
                        qh_prev = qh_new
                        e_cur = e_nxt
                        rr_cur, cr_cur = rr_nxt, cr_nxt
                    s_prev = s_new
                    d_pending = d_tiles

                # trailing diag for i = 6
                diag_half(d_pending, 0)
                diag_half(d_pending, 1)
                d_pending = None

            nc.sync.dma_start(out_ap[:, :], acc[:])

    nc.compile()
    return nc


_build_lock = threading.Lock()
_built_nc = None


def _get_nc():
    global _built_nc
    with _build_lock:
        if _built_nc is None:
            _built_nc = build()
    return _built_nc


LAST_RESULT = None  # BassKernelResults of the most recent run (for profiling)


def kernel(feats: np.ndarray) -> np.ndarray:
    global LAST_RESULT
    feats = np.ascontiguousarray(np.asarray(feats), dtype=np.float32)
    assert feats.shape == (B, C, T, N), feats.shape
    nc = _get_nc()
    in_maps = [
        {"feats": np.ascontiguousarray(feats[c * BPC:(c + 1) * BPC])}
        for c in range(NCORES)
    ]
    res = run_bass_kernel_spmd(nc, in_maps, core_ids=list(range(NCORES)))
    LAST_RESULT = res
    total = 0.0
    for r in res.results:
        total += r["out"].astype(np.float64).sum()
    loss = -total / (NSTEP * B * N)
    return np.float32(loss)


# revision 22
# speedup vs baseline: 1.3667x; 1.0131x over previous
"""Trainium2 Bass kernel for the CRW palindrome-walk contrastive loss.

Reference computation (per batch b):
  f = L2-normalize(feats, axis=C)
  A_t = f_t^T f_{t+1}                      [N,N], t = 0..T-2
  R_t = rowsoftmax(A_t / tau)              (right edges)
  L_t = rowsoftmax(A_t^T / tau)            (left edges)
  for i in 1..T-2:
    path_i = R_0 R_1 .. R_i L_i L_{i-1} .. L_0
    loss_i = -mean_n log_softmax(log(path_i + EPS))[n, n]
  loss = mean_i loss_i

Device algorithm (per core, B/8 = 2 batches), v2 restructured for engine
overlap (the fp32r baseline measured 1.32 ms with VectorE 52% busy and
the PE HAM-throttled 35% of the time):

  * All chain operands are bf16 (PE streams bf16 at the same 1 col/cycle
    as fp32r, FWL halves weight-load time, DVE ops on bf16 SBUF tiles run
    in perf mode, SBUF footprint halves).
  * Track Q_i = (R_0..R_i)^T and S_i = L_i..L_0 as in the baseline:
       Q_i = matmul(lhsT=E_i, rhs=rr_i*Q_{i-1})   (rr prescale pass)
       S_i = matmul(lhsT=E_i, rhs=S_{i-1})        (cr scale at evacuation)
    with E_t = exp(A_t/tau) kept UNnormalized; the row-softmax scale
    rr_t = 1/rowsum and col-softmax scale cr_t = 1/colsum are folded in.
  * cr_t in partition-block form comes from a SECOND affinity pass
    A_t^T = f_{t+1}^T f_t whose exp rowsums are E_t's colsums (replaces
    the baseline's 64 tiny [128,2] colsum matmuls per t).
  * Per walk step the engines split:  PE: chain + affinity + diag
    matmuls; ACT: exp(+rowsum accum) / sqrt; DVE: PSUM evacuations,
    prescales, stats; GpSimd: the diag elementwise products Q*S.
    Emission interleaves softmax(t+1) and diag(t-1) chunks into the
    chain-matmul stream so every engine has continuous work and the PE
    HAM stays at full clock.
  * diag(path_i) = colsum_k(Q_i * S_i); rows of path_i sum to 1 so
    log_softmax reduces to log(diag + EPS).
  * Each core returns the [1, N] vector of summed log-diagonals over
    (i, b); the host sums across cores in float64 and scales.
"""

import threading

import numpy as np

import concourse.bass as bass  # noqa: F401
import concourse.tile as tile
import concourse.mybir as mybir
from concourse import bacc
from concourse.bass_utils import run_bass_kernel_spmd

B, C, T, N = 16, 128, 8, 1024
NCORES = 8
BPC = B // NCORES          # batches per core
TEMP = 0.07
EPS = 1e-20
NB = N // 128              # partition blocks per matrix dim (8)
H = 512                    # PSUM half width (one bank of fp32)
NH = N // H                # halves (2)
NSTEP = T - 2              # walk steps i = 1..6

F32 = mybir.dt.float32
BF16 = mybir.dt.bfloat16
EXP = mybir.ActivationFunctionType.Exp
LN = mybir.ActivationFunctionType.Ln


def build(n_cores=NCORES, bpc=BPC):
    nc = bacc.Bacc("TRN2", target_bir_lowering=False, debug=False,
                   num_devices=n_cores)
    # Register EPS as a const AP so `activation(..., bias=EPS)` can use it.
    eps_t = nc.alloc_sbuf_tensor("const-eps", [128, 1], F32)
    nc.gpsimd.memset(eps_t.ap(), EPS)
    nc.const_aps.aps[(F32, EPS)] = eps_t.ap()
    nc.all_engine_barrier()
    feats_d = nc.dram_tensor("feats", [bpc, C, T, N], F32,
                             kind="ExternalInput")
    out_d = nc.dram_tensor("out", [1, N], F32, kind="ExternalOutput")
    feats_ap = feats_d.ap()
    out_ap = out_d.ap()

    with tile.TileContext(nc) as tc:
        with (
            # SBUF pools (bufs = ring slots per tag; [128,1024] bf16 = 2KB/p)
            tc.tile_pool(name="const", bufs=1) as const_pool,
            tc.tile_pool(name="f", bufs=2) as f_pool,        # raw fp32 slice
            tc.tile_pool(name="sq", bufs=2) as sq_pool,
            tc.tile_pool(name="nrm", bufs=4) as nrm_pool,
            tc.tile_pool(name="fh", bufs=4) as fh_pool,
            tc.tile_pool(name="e", bufs=24) as e_pool,       # E_t bf16
            tc.tile_pool(name="q", bufs=12) as q_pool,       # Q_i bf16
            tc.tile_pool(name="qh", bufs=16) as qh_pool,     # rr-prescaled Q
            tc.tile_pool(name="s", bufs=16) as s_pool,       # S_i bf16
            tc.tile_pool(name="d", bufs=8) as d_pool,        # Q*S bf16
            tc.tile_pool(name="esc", bufs=2) as esc_pool,    # A^T exp scratch
            tc.tile_pool(name="st", bufs=4) as st_pool,      # [128,NB] stats
            tc.tile_pool(name="lg", bufs=2) as lg_pool,
            tc.tile_pool(name="acc", bufs=1) as acc_pool,
            # PSUM: ps ring 2x[128,1024] (affinity/norm/diag) + chain 2+2
            tc.tile_pool(name="ps", bufs=2, space="PSUM") as ps_pool,
            tc.tile_pool(name="qps", bufs=1, space="PSUM") as qps_pool,
            tc.tile_pool(name="sps", bufs=1, space="PSUM") as sps_pool,
        ):
            ones_raw = const_pool.tile([128, 128], F32, tag="ones_raw")
            nc.vector.memset(ones_raw[:], 1.0)
            ones = const_pool.tile([128, 128], BF16, tag="ones")
            nc.vector.tensor_copy(ones[:], ones_raw[:])
            acc = acc_pool.tile([1, N], F32, tag="acc")
            nc.vector.memset(acc[:], 0.0)

            def hsl(h):
                return slice(h * H, (h + 1) * H)

            def bsl(nb):
                return slice(nb * 128, (nb + 1) * 128)

            def load_fhat(b, t):
                """DMA feats[b,:,t,:]; L2-normalize columns -> bf16 fhat."""
                f = f_pool.tile([128, N], F32, tag="f")
                nc.sync.dma_start(f[:], feats_ap[b, :, t, :])
                fh = fh_pool.tile([128, N], BF16, tag="fh")
                for h in range(NH):
                    sq = sq_pool.tile([128, H], BF16, tag="sq")
                    nc.scalar.square(sq[:], f[:, hsl(h)])
                    nps = ps_pool.tile([128, H], F32, tag="ps")
                    nc.tensor.matmul(nps[:], ones[:], sq[:],
                                     start=True, stop=True)
                    nrm = nrm_pool.tile([128, H], F32, tag="nrm")
                    nc.scalar.sqrt(nrm[:], nps[:])
                    nc.vector.reciprocal(nrm[:], nrm[:])
                    nc.vector.tensor_mul(fh[:, hsl(h)], f[:, hsl(h)], nrm[:])
                return fh

            def affinity_chunk(fhL, fhR, nb, e_dst, acc_dst):
                """One nb block of E = exp((fhL^T fhR)/tau) [128, N] bf16
                through one 2-bank PSUM tile and a single wide exp whose
                accumulator lands in acc_dst[:, nb] (full rowsum)."""
                a_ps = ps_pool.tile([128, N], F32, tag="ps", name="aps")
                for h in range(NH):
                    nc.tensor.matmul(a_ps[:, hsl(h)], fhL[:, bsl(nb)],
                                     fhR[:, hsl(h)], start=True, stop=True)
                nc.scalar.activation(
                    e_dst[:], a_ps[:], EXP, scale=1.0 / TEMP,
                    accum_out=acc_dst[:, nb:nb + 1])

            def softmax_emit(fhL, fhR, nb_pair, e_tiles, rsh, csh):
                """Emit affinity+exp for blocks nb_pair of both A (-> E
                tiles, rowsums rsh) and A^T (-> scratch, rowsums csh =
                colsums of E)."""
                for nb in nb_pair:
                    affinity_chunk(fhL, fhR, nb, e_tiles[nb], rsh)
                for nb in nb_pair:
                    esc = esc_pool.tile([128, N], BF16, tag="esc")
                    affinity_chunk(fhR, fhL, nb, esc, csh)

            def stats_finish(rsh, csh):
                """rr = 1/rowsums, cr = 1/colsums, both [128, NB]."""
                rr = st_pool.tile([128, NB], F32, tag="rr")
                nc.vector.reciprocal(rr[:], rsh[:])
                cr = st_pool.tile([128, NB], F32, tag="cr")
                nc.vector.reciprocal(cr[:], csh[:])
                return rr, cr

            def new_stat_halves(tag):
                return st_pool.tile([128, NB], F32, tag=tag,
                                    name=f"st_{tag}")

            def chain_mb(e_tiles, qh_prev, s_prev, mb):
                """Q[mb] then S[mb]: each 16 accumulating matmuls into a
                2-bank PSUM tile, evacuated by DVE to bf16 SBUF."""
                qp = qps_pool.tile([128, N], F32, tag="qps")
                for kb in range(NB):
                    for h in range(NH):
                        nc.tensor.matmul(qp[:, hsl(h)],
                                         e_tiles[kb][:, bsl(mb)],
                                         qh_prev[kb][:, hsl(h)],
                                         start=(kb == 0), stop=(kb == NB - 1))
                qn = q_pool.tile([128, N], BF16, tag="q")
                nc.vector.tensor_copy(qn[:], qp[:])

                sp = sps_pool.tile([128, N], F32, tag="sps")
                for kb in range(NB):
                    for h in range(NH):
                        nc.tensor.matmul(sp[:, hsl(h)],
                                         e_tiles[kb][:, bsl(mb)],
                                         s_prev[kb][:, hsl(h)],
                                         start=(kb == 0), stop=(kb == NB - 1))
                return qn, sp

            def diag_half(d_tiles, h):
                """colsum_k(Q*S) for one 512-half; LN(+EPS) into acc."""
                d_ps = ps_pool.tile([1, H], F32, tag="ps")
                for kb in range(NB):
                    nc.tensor.matmul(d_ps[:], ones[:, 0:1],
                                     d_tiles[kb][:, hsl(h)],
                                     start=(kb == 0), stop=(kb == NB - 1))
                lg = lg_pool.tile([1, H], F32, tag="lg")
                nc.scalar.activation(lg[:], d_ps[:], LN, bias=EPS)
                nc.vector.tensor_add(acc[:, hsl(h)], acc[:, hsl(h)], lg[:])

            for b in range(bpc):
                # ---------------- prelude: t0 + softmax(1) ----------------
                fh = {0: load_fhat(b, 0), 1: load_fhat(b, 1)}

                # E'_0 = exp(A_0^T/tau): seeds BOTH chains.
                e0 = [e_pool.tile([128, N], BF16, tag="e", name=f"e0_{kb}")
                      for kb in range(NB)]
                rs0h = new_stat_halves("r0h")
                for nb in range(NB):
                    affinity_chunk(fh[1], fh[0], nb, e0[nb], rs0h)

                # Q_0 raw = E'_0; its missing column scale 1/colsum(E'_0)
                # is folded into the loss: acc[j] -= NSTEP*log(colsum[j]).
                for h in range(NH):
                    c_ps = ps_pool.tile([1, H], F32, tag="ps")
                    for kb in range(NB):
                        nc.tensor.matmul(c_ps[:], ones[:, 0:1],
                                         e0[kb][:, hsl(h)],
                                         start=(kb == 0), stop=(kb == NB - 1))
                    lgc = lg_pool.tile([1, H], F32, tag="lg")
                    nc.scalar.activation(lgc[:], c_ps[:], LN)
                    nc.vector.tensor_scalar(
                        out=lgc[:], in0=lgc[:], scalar1=-float(NSTEP),
                        scalar2=None, op0=mybir.AluOpType.mult)
                    nc.vector.tensor_add(acc[:, hsl(h)], acc[:, hsl(h)],
                                         lgc[:])

                # softmax(1): E_1, rr_1, cr_1
                fh[2] = load_fhat(b, 2)
                e_cur = [e_pool.tile([128, N], BF16, tag="e", name=f"e1_{kb}")
                         for kb in range(NB)]
                rsh = new_stat_halves("rsh")
                csh = new_stat_halves("csh")
                softmax_emit(fh[1], fh[2], range(NB), e_cur, rsh, csh)
                rr_cur, cr_cur = stats_finish(rsh, csh)

                # rr'_0 = 1/rowsum(E'_0): S_0 = rowsoftmax(E'_0) = L_0.
                rr0 = st_pool.tile([128, NB], F32, tag="rr0")
                nc.vector.reciprocal(rr0[:], rs0h[:])
                s_prev = []
                qh_prev = []
                for kb in range(NB):
                    s0 = s_pool.tile([128, N], BF16, tag="s")
                    nc.vector.tensor_scalar_mul(s0[:], e0[kb][:],
                                                rr0[:, kb:kb + 1])
                    s_prev.append(s0)
                    # Q'_0 = rr_1-prescaled raw Q_0
                    q0 = qh_pool.tile([128, N], BF16, tag="qh")
                    nc.vector.tensor_scalar_mul(q0[:], e0[kb][:],
                                                rr_cur[:, kb:kb + 1])
                    qh_prev.append(q0)

                d_pending = None     # d tiles of step i-1 awaiting diag
                # ---------------- walk steps i = 1..6 ----------------
                for i in range(1, NSTEP + 1):
                    last = (i == NSTEP)
                    e_nxt = None
                    rsh_n = csh_n = None
                    if not last:
                        fh[i + 2] = load_fhat(b, i + 2)
                        e_nxt = [e_pool.tile([128, N], BF16, tag="e",
                                            name=f"e_{i}_{kb}")
                                 for kb in range(NB)]
                        rsh_n = new_stat_halves("rsh")
                        csh_n = new_stat_halves("csh")

                    q_new, s_new = [], []
                    for mb in range(NB):
                        qn, sp = chain_mb(e_cur, qh_prev, s_prev, mb)
                        q_new.append(qn)
                        # S evac with cr_i scale
                        sn = s_pool.tile([128, N], BF16, tag="s")
                        nc.vector.tensor_scalar_mul(sn[:], sp[:],
                                                    cr_cur[:, mb:mb + 1])
                        s_new.append(sn)
                        # interleave lookahead softmax + trailing diag into
                        # the PE stream between chain blocks
                        if not last and mb in (0, 2, 4, 6):
                            softmax_emit(fh[i + 1], fh[i + 2],
                                         (mb, mb + 1), e_nxt, rsh_n, csh_n)
                        if d_pending is not None and mb in (2, 5):
                            diag_half(d_pending, 0 if mb == 2 else 1)
                            if mb == 5:
                                d_pending = None

                    # d_i = Q_i * S_i on GpSimd (bf16, SBUF only)
                    d_tiles = []
                    for kb in range(NB):
                        d = d_pool.tile([128, N], BF16, tag="d")
                        nc.gpsimd.tensor_mul(d[:], q_new[kb][:], s_new[kb][:])
                        d_tiles.append(d)

                    if not last:
                        rr_nxt, cr_nxt = stats_finish(rsh_n, csh_n)
                        # prescale Q_i by rr_{i+1} for the next step's rhs
                        qh_new = []
                        for kb in range(NB):
                            qh = qh_pool.tile([128, N], BF16, tag="qh")
                            nc.vector.tensor_scalar_mul(
                                qh[:], q_new[kb][:], rr_nxt[:, kb:kb + 1])
                            qh_new.append(qh)
                        qh_prev = qh_new
                        e_cur = e_nxt
                        rr_cur, cr_cur = rr_nxt, cr_nxt
                    s_prev = s_new
                    d_pending = d_tiles

                # trailing diag for i = 6
                diag_half(d_pending, 0)
                diag_half(d_pending, 1)
                d_pending = None

            nc.sync.dma_start(out_ap[:, :], acc[:])

    nc.compile()
    return nc


_build_lock = threading.Lock()
_built_nc = None


def _get_nc():
    global _built_nc
    with _build_lock:
        if _built_nc is None:
            _built_nc = build()
    return _built_nc


LAST_RESULT = None  # BassKernelResults of the most recent run (for profiling)


def kernel(feats: np.ndarray) -> np.ndarray:
    global LAST_RESULT
    feats = np.ascontiguousarray(np.asarray(feats), dtype=np.float32)
    assert feats.shape == (B, C, T, N), feats.shape
    nc = _get_nc()
    in_maps = [
        {"feats": np.ascontiguousarray(feats[c * BPC:(c + 1) * BPC])}
        for c in range(NCORES)
    ]
    res = run_bass_kernel_spmd(nc, in_maps, core_ids=list(range(NCORES)))
    LAST_RESULT = res
    total = 0.0
    for r in res.results:
        total += r["out"].astype(np.float64).sum()
    loss = -total / (NSTEP * B * N)
    return np.float32(loss)
